# revision 11
# baseline (speedup 1.0000x reference)
"""Trainium2 Bass kernel for nn_DGCRM_88227218194820.

The reference module's dynamic-adjacency branch (gconv_hyper / nodevec /
adp) is dead code w.r.t. the returned hidden state: due to the faithful
source bug, gconv_rnn(inp, i) == concat([inp, a*inp, a*inp], -1) @ rnn_W[i]
+ rnn_b[i] uses no adjacency, and the normalized adjacencies are deleted.
The output therefore reduces to a per-row GRU gate:

    combined = concat(x, h)                      # [.., 66]
    z  = sigmoid(combined @ Wz + bz)
    r  = sigmoid(combined @ Wr + br)
    hc = tanh(concat(x, r*h) @ Wc + bc)
    out = z*h + (1-z)*hc

with Wg folded from rnn_W: Wg = W[:66] + a*(W[66:132] + W[132:198]),
summed over the two gconv_rnn calls per gate.

Layout (per core, data-parallel over batch: 2 of 16 batches per core,
R = 2048 rows): everything lives transposed (channels on partitions) and
"group-stacked" -- rows 0:1024 (group A) on partitions 0:64, rows
1024:2048 (group B) on partitions 64:128, so every ACT/DVE op uses all
128 partitions.  Each gate matmul uses a K=128 block-diagonal bf16
weight blockdiag(Wg_h, Wg_h), which computes both groups' pre-acts in
one instruction with PSUM output already group-stacked; the 2-channel x
contribution accumulates via a K=4 block-diagonal matmul.  Matmul inputs
are bf16 (fp32 PE matmul is ~4x slower); PSUM accumulation and the
gating arithmetic stay fp32.

Perf structure:
 - a PE warm-up burst of dummy matmuls runs while the input DMAs are in
   flight, so the HAM clock gate reaches 2.4 GHz before the real matmuls
 - input DMAs are merged (bitcast-packed) to cut HWDGE descriptor-gen
   serialization on SP, and ordered so the matmul operands land first
"""

import ml_dtypes
import numpy as np

import concourse.tile as tile
from concourse import bacc, mybir
from concourse.bass_utils import run_bass_kernel_spmd

N_CORES = 8
B, N, IN_DIM, HID = 16, 1024, 2, 64
GC_ALPHA = 0.05
CIN = HID + IN_DIM          # 66
R = (B // N_CORES) * N      # 2048 rows per core
G = R // 2                  # 1024 rows per group (A/B)
BLK = 512                   # psum free-dim block
NBLK = G // BLK             # 2
N_WARMUP_MM = 8

F32 = mybir.dt.float32
BF16 = mybir.dt.bfloat16
AF = mybir.ActivationFunctionType
BF16_NP = ml_dtypes.bfloat16

_program_cache = {}


def build_program():
    # Bacc (not raw Bass): its compile() runs move_matmul_waits_to_ldweights
    # + generate_event_semaphores, which split multi-sem waits to satisfy
    # the TRN2 "at most 1 sync wait per instruction" constraint.
    nc = bacc.Bacc()
    htb = nc.dram_tensor("htb", [128, G], BF16, kind="ExternalInput")
    # aux1: bf16 blockdiag gate weights (bitcast-packed) + f32 biases
    aux1 = nc.dram_tensor("aux1", [128, 195], F32, kind="ExternalInput")
    # aux2: bf16 blockdiag x-weights + bf16 x data (bitcast-packed)
    aux2 = nc.dram_tensor("aux2", [4, 704], F32, kind="ExternalInput")
    ht = nc.dram_tensor("ht", [128, G], F32, kind="ExternalInput")
    ot = nc.dram_tensor("ot", [128, G], F32, kind="ExternalOutput")

    with tile.TileContext(nc) as tc:
        with (
            tc.tile_pool(name="sb", bufs=1) as sb,
            tc.tile_pool(name="ps", bufs=1, space="PSUM") as ps,
        ):
            HT = sb.tile([128, G], F32, tag="HT")
            HTB = sb.tile([128, G], BF16, tag="HTB")
            AUX1 = sb.tile([128, 195], F32, tag="AUX1")
            AUX2 = sb.tile([4, 704], F32, tag="AUX2")
            ZT = sb.tile([128, G], F32, tag="ZT")
            RT = sb.tile([128, G], F32, tag="RT")
            RHB = sb.tile([128, G], BF16, tag="RHB")
            HC = sb.tile([128, G], F32, tag="HC")
            DD = sb.tile([128, G], F32, tag="DD")
            ZD = sb.tile([128, G], F32, tag="ZD")
            OT = sb.tile([128, G], F32, tag="OT")
            WARM = sb.tile([128, BLK], BF16, tag="WARM")
            dummy = sb.tile([1, 1], F32, tag="dummy")

            WB = AUX1[:, 0:192].bitcast(BF16)    # [128, 384]
            BI = AUX1[:, 192:195]                # [128, 3]
            WX = AUX2[:, 0:192].bitcast(BF16)    # [4, 384]
            XT = AUX2[:, 192:704].bitcast(BF16)  # [4, 1024]

            # Fire the ACT table load (sigmoid_and_others, covers tanh)
            # immediately so it overlaps the input DMAs.  Use an AP bias
            # to avoid constant-tensor init boilerplate.
            nc.vector.memset(dummy, 0.0)
            nc.scalar.activation(
                out=dummy, in_=dummy, func=AF.Sigmoid, bias=dummy[0:1, 0:1]
            )

            # Input DMAs, matmul operands first.  Each 2D dma_start costs
            # ~0.6us of HWDGE descriptor generation on SP, so tensors are
            # merged (bitcast-packed) into few transfers.
            nc.sync.dma_start(out=HTB, in_=htb[:, :])
            nc.sync.dma_start(out=AUX1, in_=aux1[:, :])
            nc.sync.dma_start(out=AUX2, in_=aux2[:, :])
            # ht (only needed by the DVE blend, later) goes via SWDGE on
            # gpsimd: its descriptor generation runs on the Q7 cores in
            # parallel with SP's HWDGE descriptor generation above.
            nc.gpsimd.dma_start(out=HT, in_=ht[:, :])

            # PE warm-up: dummy matmuls while DMAs are in flight keep the
            # HAM activity window busy so real matmuls run at 2.4 GHz.
            nc.vector.memset(WARM, 0.0)
            pwarm = ps.tile([128, BLK], F32, tag="pwarm")
            for _ in range(N_WARMUP_MM):
                nc.tensor.matmul(
                    pwarm[:, :], WARM[:, 0:128], WARM[:, :],
                    start=True, stop=True, skip_group_check=True,
                )

            def gate_matmuls(psum_t, g, rhs_t, cols, x_first=False):
                wc = slice(128 * g, 128 * g + 128)
                mm_h = (WB[:, wc], rhs_t[:, cols])
                mm_x = (WX[0:4, wc], XT[0:4, cols])
                first, second = (mm_x, mm_h) if x_first else (mm_h, mm_x)
                nc.tensor.matmul(
                    psum_t[:, :], first[0], first[1], start=True, stop=False
                )
                nc.tensor.matmul(
                    psum_t[:, :], second[0], second[1], start=False, stop=True
                )

            for p in range(NBLK):
                cols = slice(p * BLK, (p + 1) * BLK)
                pz = ps.tile([128, BLK], F32, tag=f"pz{p}")
                pr = ps.tile([128, BLK], F32, tag=f"pr{p}")
                gate_matmuls(pz, 0, HTB, cols)
                gate_matmuls(pr, 1, HTB, cols)
                nc.scalar.activation(
                    out=ZT[:, cols], in_=pz[:, :], func=AF.Sigmoid,
                    bias=BI[:, 0:1],
                )
                nc.scalar.activation(
                    out=RT[:, cols], in_=pr[:, :], func=AF.Sigmoid,
                    bias=BI[:, 1:2],
                )
                nc.vector.tensor_mul(RHB[:, cols], RT[:, cols], HT[:, cols])

            for p in range(NBLK):
                cols = slice(p * BLK, (p + 1) * BLK)
                pc = ps.tile([128, BLK], F32, tag=f"pc{p}")
                # x-part first: it does not depend on r*h, so only the
                # h-part matmul sits on the critical path after rh.
                gate_matmuls(pc, 2, RHB, cols, x_first=True)
                nc.scalar.activation(
                    out=HC[:, cols], in_=pc[:, :], func=AF.Tanh,
                    bias=BI[:, 2:3],
                )
                nc.vector.tensor_sub(DD[:, cols], HT[:, cols], HC[:, cols])
                nc.vector.tensor_mul(ZD[:, cols], ZT[:, cols], DD[:, cols])
                nc.vector.tensor_add(OT[:, cols], HC[:, cols], ZD[:, cols])
                nc.sync.dma_start(out=ot[:, cols], in_=OT[:, cols])

    nc.compile()
    return nc


def get_program():
    if "nc" not in _program_cache:
        _program_cache["nc"] = build_program()
    return _program_cache["nc"]


def fold_params(rnn_W, rnn_b):
    """Fold the gconv_rnn bug + gate sums into per-gate [66,64] weights."""
    Wf = rnn_W[:, :CIN, :] + GC_ALPHA * (
        rnn_W[:, CIN : 2 * CIN, :] + rnn_W[:, 2 * CIN : 3 * CIN, :]
    )  # [6, 66, 64]
    Wg = np.stack([Wf[0] + Wf[1], Wf[2] + Wf[3], Wf[4] + Wf[5]])  # [3,66,64]
    bg = np.stack(
        [rnn_b[0] + rnn_b[1], rnn_b[2] + rnn_b[3], rnn_b[4] + rnn_b[5]]
    )  # [3, 64]
    return Wg, bg


def make_in_maps(x, h, rnn_W, rnn_b):
    Wg, bg = fold_params(rnn_W, rnn_b)
    # combined = concat(x, h): channels 0:2 are x, 2:66 are h
    W_x = Wg[:, :IN_DIM, :]  # [3, 2, 64]
    W_h = Wg[:, IN_DIM:, :]  # [3, 64, 64]

    # Block-diagonal bf16 weights: gate g occupies cols 128g:128(g+1);
    # out = blockdiag(Wg_h, Wg_h).T @ [h_A; h_B] = [gate_A; gate_B].
    wb_host = np.zeros((128, 384), BF16_NP)
    wx_host = np.zeros((4, 384), BF16_NP)
    for g in range(3):
        wb_host[0:64, 128 * g : 128 * g + 64] = W_h[g]
        wb_host[64:128, 128 * g + 64 : 128 * g + 128] = W_h[g]
        wx_host[0:2, 128 * g : 128 * g + 64] = W_x[g]
        wx_host[2:4, 128 * g + 64 : 128 * g + 128] = W_x[g]
    bihalf = bg.T  # [64, 3]
    bi_host = np.concatenate([bihalf, bihalf], axis=0)  # [128, 3]

    aux1_host = np.empty((128, 195), np.float32)
    aux1_host[:, 0:192] = wb_host.view(np.float32)
    aux1_host[:, 192:195] = bi_host

    hf = h.reshape(N_CORES, R, HID)
    xf = x.reshape(N_CORES, R, IN_DIM)
    in_maps = []
    for c in range(N_CORES):
        ht_host = np.ascontiguousarray(
            np.concatenate([hf[c, :G].T, hf[c, G:].T], axis=0)
        )  # [128, G] f32
        xt_host = np.ascontiguousarray(
            np.concatenate([xf[c, :G].T, xf[c, G:].T], axis=0)
        ).astype(BF16_NP)  # [4, G]
        aux2_host = np.empty((4, 704), np.float32)
        aux2_host[:, 0:192] = wx_host.view(np.float32)
        aux2_host[:, 192:704] = xt_host.view(np.float32)
        in_maps.append(
            dict(
                htb=ht_host.astype(BF16_NP),
                aux1=aux1_host,
                aux2=aux2_host,
                ht=ht_host,
            )
        )
    return in_maps


def gather_output(results):
    outs = []
    for c in range(N_CORES):
        o = np.asarray(results[c]["ot"])  # [128, G]
        outs.append(np.concatenate([o[:64].T, o[64:].T], axis=0))  # [R, HID]
    return (
        np.concatenate(outs, axis=0).reshape(B, N, HID).astype(np.float32)
    )


def run(inputs, trace=False, **kw):
    x = np.ascontiguousarray(np.asarray(inputs["x"], dtype=np.float32))
    h = np.ascontiguousarray(
        np.asarray(inputs["hidden_state"], dtype=np.float32)
    )
    rnn_W = np.asarray(inputs["rnn_W"], dtype=np.float32)
    rnn_b = np.asarray(inputs["rnn_b"], dtype=np.float32)

    in_maps = make_in_maps(x, h, rnn_W, rnn_b)
    nc = get_program()
    res = run_bass_kernel_spmd(
        nc, in_maps, core_ids=list(range(N_CORES)), trace=trace, **kw
    )
    return gather_output(res.results), res


def kernel(**inputs) -> np.ndarray:
    out, _ = run(inputs)
    return out


# revision 12
# speedup vs baseline: 1.1064x; 1.1064x over previous
"""Trainium2 Bass kernel for nn_DGCRM_88227218194820.

The reference module's dynamic-adjacency branch (gconv_hyper / nodevec /
adp) is dead code w.r.t. the returned hidden state: due to the faithful
source bug, gconv_rnn(inp, i) == concat([inp, a*inp, a*inp], -1) @ rnn_W[i]
+ rnn_b[i] uses no adjacency, and the normalized adjacencies are deleted.
The output therefore reduces to a per-row GRU gate:

    combined = concat(x, h)                      # [.., 66]
    z  = sigmoid(combined @ Wz + bz)
    r  = sigmoid(combined @ Wr + br)
    hc = tanh(concat(x, r*h) @ Wc + bc)
    out = z*h + (1-z)*hc

with Wg folded from rnn_W: Wg = W[:66] + a*(W[66:132] + W[132:198]),
summed over the two gconv_rnn calls per gate.

Layout (per core, data-parallel over batch: 2 of 16 batches per core,
R = 2048 rows): everything lives transposed (channels on partitions) and
"group-stacked" -- rows 0:1024 (group A) on partitions 0:64, rows
1024:2048 (group B) on partitions 64:128, so every ACT/DVE op uses all
128 partitions.  Each gate matmul uses a K=128 block-diagonal bf16
weight blockdiag(Wg_h, Wg_h), which computes both groups' pre-acts in
one instruction with PSUM output already group-stacked; the 2-channel x
contribution accumulates via a K=4 block-diagonal matmul.

dtypes: matmul inputs bf16 (fp32 PE matmul is ~4x slower), PSUM
accumulation fp32, activations + gating arithmetic bf16 (fp32
tensor_tensor on the DVE has no fast mode; bf16 runs 2x), output bf16
(upcast on host).  Measured end-to-end relative error ~4e-3.

Perf structure:
 - a PE warm-up burst of dummy matmuls runs while the input DMAs are in
   flight, so the HAM clock gate reaches 2.4 GHz before the real matmuls
 - input DMAs are merged (bitcast-packed) into three transfers to cut
   HWDGE descriptor-gen serialization on SP, matmul operands first
"""

import ml_dtypes
import numpy as np

import concourse.tile as tile
from concourse import bacc, mybir
from concourse.bass_utils import run_bass_kernel_spmd

N_CORES = 8
B, N, IN_DIM, HID = 16, 1024, 2, 64
GC_ALPHA = 0.05
CIN = HID + IN_DIM          # 66
R = (B // N_CORES) * N      # 2048 rows per core
G = R // 2                  # 1024 rows per group (A/B)
BLK = 512                   # psum free-dim block
NBLK = G // BLK             # 2
N_WARMUP_MM = 6

F32 = mybir.dt.float32
BF16 = mybir.dt.bfloat16
AF = mybir.ActivationFunctionType
BF16_NP = ml_dtypes.bfloat16

_program_cache = {}


def build_program():
    # Bacc (not raw Bass): its compile() runs move_matmul_waits_to_ldweights
    # + generate_event_semaphores, which split multi-sem waits to satisfy
    # the TRN2 "at most 1 sync wait per instruction" constraint.
    nc = bacc.Bacc()
    # aux1: bf16 blockdiag gate weights (bitcast-packed) + f32 biases
    aux1 = nc.dram_tensor("aux1", [128, 195], F32, kind="ExternalInput")
    htb = nc.dram_tensor("htb", [128, G], BF16, kind="ExternalInput")
    # aux2: bf16 blockdiag x-weights + bf16 x data (bitcast-packed)
    aux2 = nc.dram_tensor("aux2", [4, 704], F32, kind="ExternalInput")
    ot = nc.dram_tensor("ot", [128, G], BF16, kind="ExternalOutput")

    with tile.TileContext(nc) as tc:
        with (
            tc.tile_pool(name="sb", bufs=1) as sb,
            tc.tile_pool(name="ps", bufs=1, space="PSUM") as ps,
        ):
            HTB = sb.tile([128, G], BF16, tag="HTB")
            AUX1 = sb.tile([128, 195], F32, tag="AUX1")
            AUX2 = sb.tile([4, 704], F32, tag="AUX2")
            ZT = sb.tile([128, G], BF16, tag="ZT")
            RT = sb.tile([128, G], BF16, tag="RT")
            RHB = sb.tile([128, G], BF16, tag="RHB")
            HC = sb.tile([128, G], BF16, tag="HC")
            DD = sb.tile([128, G], BF16, tag="DD")
            ZD = sb.tile([128, G], BF16, tag="ZD")
            OT = sb.tile([128, G], BF16, tag="OT")
            WARM = sb.tile([128, BLK], BF16, tag="WARM")
            dummy = sb.tile([1, 1], F32, tag="dummy")

            WB = AUX1[:, 0:192].bitcast(BF16)    # [128, 384]
            BI = AUX1[:, 192:195]                # [128, 3]
            WX = AUX2[:, 0:192].bitcast(BF16)    # [4, 384]
            XT = AUX2[:, 192:704].bitcast(BF16)  # [4, 1024]

            # Fire the ACT table load (sigmoid_and_others, covers tanh)
            # immediately so it overlaps the input DMAs.  Use an AP bias
            # to avoid constant-tensor init boilerplate.
            nc.vector.memset(dummy, 0.0)
            nc.scalar.activation(
                out=dummy, in_=dummy, func=AF.Sigmoid, bias=dummy[0:1, 0:1]
            )

            # Input DMAs, matmul operands first.  Each 2D dma_start costs
            # ~0.6us of HWDGE descriptor generation on SP, so tensors are
            # merged (bitcast-packed) into few transfers.
            nc.sync.dma_start(out=AUX1, in_=aux1[:, :])
            nc.sync.dma_start(out=HTB, in_=htb[:, :])
            nc.sync.dma_start(out=AUX2, in_=aux2[:, :])

            # PE warm-up: dummy matmuls while DMAs are in flight keep the
            # HAM activity window busy so real matmuls run at 2.4 GHz.
            nc.vector.memset(WARM, 0.0)
            pwarm = ps.tile([128, BLK], F32, tag="pwarm")
            for _ in range(N_WARMUP_MM):
                nc.tensor.matmul(
                    pwarm[:, :], WARM[:, 0:128], WARM[:, :],
                    start=True, stop=True, skip_group_check=True,
                )

            def gate_matmuls(psum_t, g, rhs_t, cols, x_first=False):
                wc = slice(128 * g, 128 * g + 128)
                mm_h = (WB[:, wc], rhs_t[:, cols])
                mm_x = (WX[0:4, wc], XT[0:4, cols])
                first, second = (mm_x, mm_h) if x_first else (mm_h, mm_x)
                nc.tensor.matmul(
                    psum_t[:, :], first[0], first[1], start=True, stop=False
                )
                nc.tensor.matmul(
                    psum_t[:, :], second[0], second[1], start=False, stop=True
                )

            for p in range(NBLK):
                cols = slice(p * BLK, (p + 1) * BLK)
                pz = ps.tile([128, BLK], F32, tag=f"pz{p}")
                pr = ps.tile([128, BLK], F32, tag=f"pr{p}")
                gate_matmuls(pz, 0, HTB, cols)
                gate_matmuls(pr, 1, HTB, cols)
                nc.scalar.activation(
                    out=ZT[:, cols], in_=pz[:, :], func=AF.Sigmoid,
                    bias=BI[:, 0:1],
                )
                nc.scalar.activation(
                    out=RT[:, cols], in_=pr[:, :], func=AF.Sigmoid,
                    bias=BI[:, 1:2],
                )
                nc.vector.tensor_mul(RHB[:, cols], RT[:, cols], HTB[:, cols])

            for p in range(NBLK):
                cols = slice(p * BLK, (p + 1) * BLK)
                pc = ps.tile([128, BLK], F32, tag=f"pc{p}")
                # x-part first: it does not depend on r*h, so only the
                # h-part matmul sits on the critical path after rh.
                gate_matmuls(pc, 2, RHB, cols, x_first=True)
                nc.scalar.activation(
                    out=HC[:, cols], in_=pc[:, :], func=AF.Tanh,
                    bias=BI[:, 2:3],
                )
                nc.vector.tensor_sub(DD[:, cols], HTB[:, cols], HC[:, cols])
                nc.vector.tensor_mul(ZD[:, cols], ZT[:, cols], DD[:, cols])
                nc.vector.tensor_add(OT[:, cols], HC[:, cols], ZD[:, cols])
                nc.sync.dma_start(out=ot[:, cols], in_=OT[:, cols])

    nc.compile()
    return nc


def get_program():
    if "nc" not in _program_cache:
        _program_cache["nc"] = build_program()
    return _program_cache["nc"]


def fold_params(rnn_W, rnn_b):
    """Fold the gconv_rnn bug + gate sums into per-gate [66,64] weights."""
    Wf = rnn_W[:, :CIN, :] + GC_ALPHA * (
        rnn_W[:, CIN : 2 * CIN, :] + rnn_W[:, 2 * CIN : 3 * CIN, :]
    )  # [6, 66, 64]
    Wg = np.stack([Wf[0] + Wf[1], Wf[2] + Wf[3], Wf[4] + Wf[5]])  # [3,66,64]
    bg = np.stack(
        [rnn_b[0] + rnn_b[1], rnn_b[2] + rnn_b[3], rnn_b[4] + rnn_b[5]]
    )  # [3, 64]
    return Wg, bg


def make_in_maps(x, h, rnn_W, rnn_b):
    Wg, bg = fold_params(rnn_W, rnn_b)
    # combined = concat(x, h): channels 0:2 are x, 2:66 are h
    W_x = Wg[:, :IN_DIM, :]  # [3, 2, 64]
    W_h = Wg[:, IN_DIM:, :]  # [3, 64, 64]

    # Block-diagonal bf16 weights: gate g occupies cols 128g:128(g+1);
    # out = blockdiag(Wg_h, Wg_h).T @ [h_A; h_B] = [gate_A; gate_B].
    wb_host = np.zeros((128, 384), BF16_NP)
    wx_host = np.zeros((4, 384), BF16_NP)
    for g in range(3):
        wb_host[0:64, 128 * g : 128 * g + 64] = W_h[g]
        wb_host[64:128, 128 * g + 64 : 128 * g + 128] = W_h[g]
        wx_host[0:2, 128 * g : 128 * g + 64] = W_x[g]
        wx_host[2:4, 128 * g + 64 : 128 * g + 128] = W_x[g]
    bihalf = bg.T  # [64, 3]
    bi_host = np.concatenate([bihalf, bihalf], axis=0)  # [128, 3]

    aux1_host = np.empty((128, 195), np.float32)
    aux1_host[:, 0:192] = wb_host.view(np.float32)
    aux1_host[:, 192:195] = bi_host

    hf = h.reshape(N_CORES, R, HID)
    xf = x.reshape(N_CORES, R, IN_DIM)
    in_maps = []
    for c in range(N_CORES):
        ht_host = np.ascontiguousarray(
            np.concatenate([hf[c, :G].T, hf[c, G:].T], axis=0)
        )  # [128, G] f32
        xt_host = np.ascontiguousarray(
            np.concatenate([xf[c, :G].T, xf[c, G:].T], axis=0)
        ).astype(BF16_NP)  # [4, G]
        aux2_host = np.empty((4, 704), np.float32)
        aux2_host[:, 0:192] = wx_host.view(np.float32)
        aux2_host[:, 192:704] = xt_host.view(np.float32)
        in_maps.append(
            dict(
                htb=ht_host.astype(BF16_NP),
                aux1=aux1_host,
                aux2=aux2_host,
            )
        )
    return in_maps


def gather_output(results):
    outs = []
    for c in range(N_CORES):
        o = np.asarray(results[c]["ot"]).astype(np.float32)  # [128, G]
        outs.append(np.concatenate([o[:64].T, o[64:].T], axis=0))  # [R, HID]
    return (
        np.concatenate(outs, axis=0).reshape(B, N, HID).astype(np.float32)
    )


def run(inputs, trace=False, **kw):
    x = np.ascontiguousarray(np.asarray(inputs["x"], dtype=np.float32))
    h = np.ascontiguousarray(
        np.asarray(inputs["hidden_state"], dtype=np.float32)
    )
    rnn_W = np.asarray(inputs["rnn_W"], dtype=np.float32)
    rnn_b = np.asarray(inputs["rnn_b"], dtype=np.float32)

    in_maps = make_in_maps(x, h, rnn_W, rnn_b)
    nc = get_program()
    res = run_bass_kernel_spmd(
        nc, in_maps, core_ids=list(range(N_CORES)), trace=trace, **kw
    )
    return gather_output(res.results), res


def kernel(**inputs) -> np.ndarray:
    out, _ = run(inputs)
    return out
